# revision 15
# baseline (speedup 1.0000x reference)
"""Trainium2 Bass kernel for nn_CombineGraph (GNN message passing).

Strategy (8 NeuronCores, data-parallel over batch B=64 -> 8 per core):
  - Embedding table replicated per core. The dominant cost is gathering
    ~74K embedding rows per core; done with the Ant extended dma_gather
    (bf16, 256B rows). Its indices are int16 (max 32767) but V=50000, so
    the table is split into LO [0,32767) and HI [32767,50000) halves, each
    with a planted all-zero row; every slot is gathered from both halves
    (the "wrong" half yields the zero row) and the two results are added.
  - Row-major gathered tiles feed the per-group weighted-sum matmuls
    (attention application); a second SBUF-source transposing dma_gather
    (iota indices) produces the [D, n] layout for the d-contraction
    matmuls (the GlobalAggregator MLPs) on the PE in bf16.
  - Softmax over the 12 samples per group runs in a [groups, 12] layout
    produced by small AP-reshape DMAs; no max-subtraction is needed
    (logits are O(0.3) by construction).
  - The LocalAggregator (masked 4-relation GAT) runs in exact fp32.
  - Neighbor-sample index chains (adj_all[inputs] etc.) are pure integer
    lookups of host-visible inputs; they are precomputed host-side and
    shipped as gather offset tensors (input staging).
"""

import numpy as np
import ml_dtypes

import concourse.bass as bass
import concourse.bacc as bacc
import concourse.mybir as mybir
import concourse.tile as tile
from concourse.bass_utils import run_bass_kernel_spmd
from concourse.masks import make_identity

B, S, SAMPLE, D, V, HOP = 64, 64, 12, 128, 50000, 2
NCORES = 8
BL = B // NCORES          # 8 batches per core
N1 = S * SAMPLE           # 768
N2 = N1 * SAMPLE          # 9216
SPLIT = 32767             # LO covers [0, SPLIT); HI covers [SPLIT, V)
NHI = V - SPLIT + 1       # HI table rows (incl. zero row at 0)
F32 = mybir.dt.float32
BF16 = mybir.dt.bfloat16
I32 = mybir.dt.int32
I16 = mybir.dt.int16
BIG = 1.0e30

_compiled = {}


def _wrap16(v):
    """int16 idx buffer: [128, n/16]; buf[j, i] = v[16*i + j], replicated 8x."""
    n = v.shape[0]
    w = v.astype(np.int16).reshape(n // 16, 16).T
    return np.ascontiguousarray(np.tile(w, (8, 1)))


def _slotwrap(v, cols):
    """[cols*128] -> [128, cols] with slot r=(c*128+p) -> [p, c]."""
    return np.ascontiguousarray(v.reshape(cols, 128).T)


def _build_pattern():
    """PAT[128, 72*12] f32: PAT[p, c*12+k] = 1 iff (128c+p)//12 == (128c)//12 + k."""
    pat = np.zeros((128, 72 * 12), np.float32)
    for c in range(72):
        g0 = (128 * c) // 12
        for p in range(128):
            g = (128 * c + p) // 12
            pat[p, c * 12 + (g - g0)] = 1.0
    return pat


def _ws_ranges(ncol):
    """per tile-col c: (g0, gcnt) group range covered by rows 128c..128c+127."""
    out = []
    for c in range(ncol):
        g0 = (128 * c) // 12
        g1 = (128 * c + 127) // 12
        out.append((g0, g1 - g0 + 1))
    return out


def _build_program():
    nc = bacc.Bacc(None, target_bir_lowering=False)
    dt = nc.dram_tensor
    # ---- replicated tables / weights ----
    t_embf = dt("embf", [V, D], F32, kind="ExternalInput")
    t_embb = dt("embb", [V, D], BF16, kind="ExternalInput")
    t_pat = dt("pat", [128, 864], F32, kind="ExternalInput")
    t_acol = dt("acol", [128, 4], F32, kind="ExternalInput")
    t_onesh = dt("onesh", [128, 2], F32, kind="ExternalInput")
    t_w1g = dt("w1g", [128, 2 * D], F32, kind="ExternalInput")      # g0_w1[:D] | g1_w1[:D]
    t_w1d = dt("w1d", [1, 2 * D], BF16, kind="ExternalInput")       # g0_w1[D] | g1_w1[D]
    t_w2c = dt("w2c", [128, 2], BF16, kind="ExternalInput")         # g0_w2 | g1_w2
    t_w3 = dt("w3", [128, 4 * D], BF16, kind="ExternalInput")       # a0|b0|a1|b1 blocks
    # ---- per-core data ----
    t_offh = dt("offh", [128, 4], I32, kind="ExternalInput")
    t_offi = dt("offi", [128, 4], I32, kind="ExternalInput")
    t_maskf = dt("maskf", [128, 4], F32, kind="ExternalInput")
    t_adjw = dt("adjw", [128, 4 * 64], I32, kind="ExternalInput")
    t_o1 = dt("o1w", [128, 6 * BL], I32, kind="ExternalInput")
    t_o2 = dt("o2w", [128, 72 * BL], I32, kind="ExternalInput")
    t_w1v = dt("w1v", [1, BL * N1], BF16, kind="ExternalInput")
    t_w2v = dt("w2v", [1, BL * N2], BF16, kind="ExternalInput")
    t_out = dt("out", [BL * S, D], F32, kind="ExternalOutput")

    from contextlib import ExitStack
    with tile.TileContext(nc) as tc, ExitStack() as _es:
        cpool = _es.enter_context(tc.tile_pool(name="consts", bufs=1))
        pat_sb = cpool.tile([128, 864], F32)
        acol_sb = cpool.tile([128, 4], F32)
        onesh_sb = cpool.tile([128, 2], F32)
        w1g_sb = cpool.tile([128, 2 * D], F32)
        w1d_sb = cpool.tile([1, 2 * D], BF16)
        w2c_sb = cpool.tile([128, 2], BF16)
        w3_sb = cpool.tile([128, 4 * D], BF16)
        identf = cpool.tile([128, 128], F32)
        identb = cpool.tile([128, 128], BF16)
        offh_sb = cpool.tile([128, 4], I32)
        offi_sb = cpool.tile([128, 4], I32)
        maskf_sb = cpool.tile([128, 4], F32)
        adjw_sb = cpool.tile([128, 256], I32)
        o1_sb = cpool.tile([128, 6 * BL], I32)
        o2_sb = cpool.tile([128, 72 * BL], I32)
        for dst, src in [(pat_sb, t_pat), (acol_sb, t_acol),
                         (onesh_sb, t_onesh), (w1g_sb, t_w1g), (w1d_sb, t_w1d),
                         (w2c_sb, t_w2c), (w3_sb, t_w3), (offh_sb, t_offh),
                         (offi_sb, t_offi), (maskf_sb, t_maskf), (adjw_sb, t_adjw),
                         (o1_sb, t_o1), (o2_sb, t_o2)]:
            nc.sync.dma_start(dst[:], src[:])
        make_identity(nc, identf[:])
        make_identity(nc, identb[:])

        # persistent working tiles
        ppool = _es.enter_context(tc.tile_pool(name="persist", bufs=1))
        h_sb = ppool.tile([128, 4 * D], F32)          # h rows, slot layout
        item_sb = ppool.tile([128, 4 * D], F32)
        hT_sb = ppool.tile([128, BL * S], F32)        # hT per b at cols b*64..
        hTb_sb = ppool.tile([128, BL * S], BF16)
        hl_sb = ppool.tile([128, 4 * D], F32)         # h_local, slot layout
        w1s_sb = ppool.tile([128, 2 * BL * D], BF16)  # W1s per (hop, b)
        scol_sb = ppool.tile([128, BL], F32)          # s_b column vectors
        srow_sb = ppool.tile([2, 4 * D], F32)         # s_b row form (pack 2)
        recips_sb = ppool.tile([2, 4], F32)

        # ================= Phase 0: gathers + local aggregator =================
        with tc.tile_pool(name="p0psum1", bufs=1, space="PSUM") as ps0a, \
             tc.tile_pool(name="p0psum2", bufs=2, space="PSUM") as ps0, \
             tc.tile_pool(name="p0sb", bufs=2) as sb0:
            for c in range(4):
                nc.gpsimd.indirect_dma_start(
                    out=h_sb[:].rearrange("p (c d) -> p c d", c=4)[:, c, :],
                    out_offset=None, in_=t_embf[:],
                    in_offset=bass.IndirectOffsetOnAxis(ap=offh_sb[:, c:c + 1], axis=0))
                nc.gpsimd.indirect_dma_start(
                    out=item_sb[:].rearrange("p (c d) -> p c d", c=4)[:, c, :],
                    out_offset=None, in_=t_embf[:],
                    in_offset=bass.IndirectOffsetOnAxis(ap=offi_sb[:, c:c + 1], axis=0))

            # --- session summary s_b = masked mean of item rows ---
            mitem = sb0.tile([128, 4 * D], F32, tag="mitem")
            nc.vector.tensor_tensor(
                out=mitem[:].rearrange("p (c d) -> p c d", c=4),
                in0=item_sb[:].rearrange("p (c d) -> p c d", c=4),
                in1=maskf_sb[:].to_broadcast([128, 4, D]),
                op=mybir.AluOpType.mult)
            ps_s = ps0a.tile([2, 4 * D], F32, space="PSUM", tag="pss")
            ps_m = ps0a.tile([2, 4], F32, space="PSUM", tag="psm")
            nc.tensor.matmul(ps_m[:], onesh_sb[:], maskf_sb[:], start=True, stop=True)
            nc.tensor.matmul(ps_s[:], onesh_sb[:], mitem[:], start=True, stop=True)
            nc.vector.reciprocal(recips_sb[:], ps_m[:])
            nc.vector.tensor_tensor(
                out=srow_sb[:].rearrange("p (c d) -> p c d", c=4),
                in0=ps_s[:].rearrange("p (c d) -> p c d", c=4),
                in1=recips_sb[:].to_broadcast([2, 4, D]),
                op=mybir.AluOpType.mult)
            for b in range(BL):
                nc.sync.dma_start(
                    scol_sb[:, b:b + 1],
                    srow_sb[b % 2:b % 2 + 1, (b // 2) * D:(b // 2 + 1) * D])
            for hop in range(2):
                for b in range(BL):
                    nc.vector.tensor_scalar(
                        out=w1s_sb[:, (hop * BL + b) * D:(hop * BL + b + 1) * D],
                        in0=w1g_sb[:, hop * D:(hop + 1) * D],
                        scalar1=scol_sb[:, b:b + 1], scalar2=None,
                        op0=mybir.AluOpType.mult)

            # --- local aggregator, exact fp32, per b ---
            hb64s = []
            for b in range(BL):
                pb, cb = (b % 2) * 64, b // 2
                hrow = h_sb[:].rearrange("p (c d) -> p c d", c=4)[pb:pb + 64, cb, :]
                hb64 = sb0.tile([64, 128], F32, tag=f"hb64_{b}")
                nc.vector.tensor_copy(hb64[:], hrow)
                hb64s.append(hb64)
                psT = ps0.tile([128, 128], F32, space="PSUM", tag="psT")
                nc.tensor.transpose(psT[:, :64], hb64[:], identf[:64, :64])
                nc.vector.tensor_copy(hT_sb[:, b * 64:(b + 1) * 64], psT[:, :64])
                nc.scalar.copy(hTb_sb[:, b * 64:(b + 1) * 64], psT[:, :64])
            for b in range(BL):
                pb, cb = (b % 2) * 64, b // 2
                hT_b = hT_sb[:, b * 64:(b + 1) * 64]
                e_ps = ps0.tile([64, 256], F32, space="PSUM", tag="eps")
                hta = sb0.tile([128, 256], F32, tag="hta")
                for k in range(4):
                    nc.vector.tensor_scalar(
                        out=hta[:, k * 64:(k + 1) * 64], in0=hT_b,
                        scalar1=acol_sb[:, k:k + 1], scalar2=None,
                        op0=mybir.AluOpType.mult)
                for k in range(4):
                    nc.tensor.matmul(e_ps[:, k * 64:(k + 1) * 64],
                                     hta[:, k * 64:(k + 1) * 64], hT_b,
                                     start=True, stop=True)
                el = sb0.tile([64, 256], F32, tag="el")
                nc.scalar.activation(el[:], e_ps[:], mybir.ActivationFunctionType.Lrelu,
                                     alpha=0.2)
                adj_b = adjw_sb[pb:pb + 64, cb * 64:(cb + 1) * 64]
                mk = sb0.tile([64, 256], F32, tag="mk")
                for k in range(4):
                    nc.vector.tensor_scalar(out=mk[:, k * 64:(k + 1) * 64], in0=adj_b,
                                            scalar1=float(k + 1), scalar2=None,
                                            op0=mybir.AluOpType.is_equal)
                pr = sb0.tile([64, 256], F32, tag="pr")
                nc.vector.tensor_tensor(out=pr[:], in0=el[:], in1=mk[:],
                                        op=mybir.AluOpType.mult)
                alpha = sb0.tile([64, 64], F32, tag="alpha")
                nc.vector.tensor_tensor(out=alpha[:], in0=pr[:, 0:64], in1=pr[:, 64:128],
                                        op=mybir.AluOpType.add)
                nc.vector.tensor_tensor(out=alpha[:], in0=alpha[:], in1=pr[:, 128:192],
                                        op=mybir.AluOpType.add)
                nc.vector.tensor_tensor(out=alpha[:], in0=alpha[:], in1=pr[:, 192:256],
                                        op=mybir.AluOpType.add)
                pen = sb0.tile([64, 64], F32, tag="pen")
                nc.vector.tensor_scalar(out=pen[:], in0=adj_b, scalar1=0.0, scalar2=None,
                                        op0=mybir.AluOpType.is_gt)
                nc.vector.tensor_scalar(out=pen[:], in0=pen[:], scalar1=1.0, scalar2=BIG,
                                        op0=mybir.AluOpType.subtract,
                                        op1=mybir.AluOpType.mult)
                nc.vector.tensor_tensor(out=alpha[:], in0=alpha[:], in1=pen[:],
                                        op=mybir.AluOpType.add)
                ae = sb0.tile([64, 64], F32, tag="ae")
                nc.scalar.activation(ae[:], alpha[:], mybir.ActivationFunctionType.Exp)
                s1 = sb0.tile([64, 1], F32, tag="s1")
                nc.vector.reduce_sum(out=s1[:], in_=ae[:], axis=mybir.AxisListType.X)
                nc.vector.reciprocal(s1[:], s1[:])
                nc.vector.tensor_scalar(out=ae[:], in0=ae[:], scalar1=s1[:],
                                        scalar2=None, op0=mybir.AluOpType.mult)
                psT2 = ps0.tile([128, 128], F32, space="PSUM", tag="psT")
                nc.tensor.transpose(psT2[:64, :64], ae[:], identf[:64, :64])
                alphaT = sb0.tile([64, 64], F32, tag="alphaT")
                nc.vector.tensor_copy(alphaT[:], psT2[:64, :64])
                psHL = ps0.tile([64, 128], F32, space="PSUM", tag="psHL")
                nc.tensor.matmul(psHL[:], alphaT[:], hb64s[b][:],
                                 start=True, stop=True)
                nc.vector.tensor_copy(
                    hl_sb[:].rearrange("p (c d) -> p c d", c=4)[pb:pb + 64, cb, :], psHL[:])

        # ================= Phase 2: sampling gathers + global aggregator =======
        WR1 = _ws_ranges(6)
        WR2 = _ws_ranges(72)
        with tc.tile_pool(name="psA", bufs=2, space="PSUM") as psA, \
             tc.tile_pool(name="psE", bufs=2, space="PSUM") as psE, \
             tc.tile_pool(name="psNV", bufs=1, space="PSUM") as psNV, \
             tc.tile_pool(name="psTb", bufs=2, space="PSUM") as psTb, \
             tc.tile_pool(name="g1", bufs=2) as g1p, \
             tc.tile_pool(name="g2", bufs=2) as g2p, \
             tc.tile_pool(name="sc", bufs=3) as scp, \
             tc.tile_pool(name="drp", bufs=2, space="DRAM") as drp:
            for b in range(BL):
                # ---------- gathers for this b ----------
                r1lo = g1p.tile([128, 6 * D], BF16, tag="r1lo")
                r2lo = g1p.tile([128, 72 * D], BF16, tag="r2lo")
                rt1 = g2p.tile([128, N1], BF16, tag="rt1")
                rt2 = g2p.tile([128, N2], BF16, tag="rt2")
                for c in range(6):
                    nc.gpsimd.indirect_dma_start(
                        out=r1lo[:].rearrange("p (c d) -> p c d", c=6)[:, c, :],
                        out_offset=None, in_=t_embb[:],
                        in_offset=bass.IndirectOffsetOnAxis(
                            ap=o1_sb[:, b * 6 + c:b * 6 + c + 1], axis=0))
                for c in range(72):
                    nc.gpsimd.indirect_dma_start(
                        out=r2lo[:].rearrange("p (c d) -> p c d", c=72)[:, c, :],
                        out_offset=None, in_=t_embb[:],
                        in_offset=bass.IndirectOffsetOnAxis(
                            ap=o2_sb[:, b * 72 + c:b * 72 + c + 1], axis=0))
                for c in range(6):
                    nc.sync.dma_start(out=rt1[:, c * 128:(c + 1) * 128],
                                      in_=r1lo[:, c * D:(c + 1) * D], transpose=True)
                for c in range(72):
                    nc.sync.dma_start(out=rt2[:, c * 128:(c + 1) * 128],
                                      in_=r2lo[:, c * D:(c + 1) * D], transpose=True)

                # ---------- hop stages ----------
                ev1T = scp.tile([128, N1], BF16, tag="ev1T")      # relu out of h1
                ev0T = scp.tile([128, 64], BF16, tag="ev0T")      # relu out of h0
                r1g = scp.tile([128, 6 * D], BF16, tag="r1g")     # rows of ev1T
                hgT = scp.tile([128, 64], BF16, tag="hgT")

                stages = [
                    ("h0", 0, rt1, r1lo, t_w1v, b * N1, N1, 6, WR1, hTb_sb[:, b * 64:(b + 1) * 64], 64, ev0T),
                    ("h1", 0, rt2, r2lo, t_w2v, b * N2, N2, 72, WR2, rt1[:], N1, ev1T),
                    ("g1", 1, ev1T[:], r1g, t_w1v, b * N1, N1, 6, WR1, ev0T[:], 64, hgT),
                ]
                for (nm, hop, rtT, rows, wv, wv0, n, ncol, wr, evT, M, outt) in stages:
                    rtT_ap = rtT if isinstance(rtT, bass.AP) else rtT[:]
                    if nm == "g1":
                        # rows of ev1T via PE transposes
                        for tix in range(6):
                            psTT = psTb.tile([128, 128], BF16, space="PSUM", tag="psTb")
                            nc.tensor.transpose(psTT[:],
                                                ev1T[:, tix * 128:(tix + 1) * 128],
                                                identb[:])
                            nc.vector.tensor_copy(
                                r1g[:].rearrange("p (c d) -> p c d", c=6)[:, tix, :],
                                psTT[:])
                    w1s_b = w1s_sb[:, (hop * BL + b) * D:(hop * BL + b + 1) * D]
                    w1d_h = w1d_sb[:, hop * D:(hop + 1) * D]
                    w2c_h = w2c_sb[:, hop:hop + 1]
                    w3a = w3_sb[:, (2 * hop) * D:(2 * hop + 1) * D]
                    w3b = w3_sb[:, (2 * hop + 1) * D:(2 * hop + 2) * D]
                    # ---- A = W1s^T rtT + w1D (x) wv ; leaky ; e = w2^T LAT ----
                    efd = drp.tile([1, N2], F32, tag="efd")
                    nchunks = (n + 511) // 512
                    for ch in range(nchunks):
                        c0, cn = ch * 512, min(512, n - ch * 512)
                        wvc = scp.tile([1, 512], BF16, tag="wvc")
                        nc.sync.dma_start(wvc[:, :cn], wv[:, wv0 + c0:wv0 + c0 + cn])
                        psa = psA.tile([128, 512], F32, space="PSUM", tag="psA")
                        nc.tensor.matmul(psa[:, :cn], w1s_b, rtT_ap[:, c0:c0 + cn],
                                         start=True, stop=False)
                        nc.tensor.matmul(psa[:, :cn], w1d_h, wvc[:, :cn],
                                         start=False, stop=True)
                        lat = scp.tile([128, 512], BF16, tag="lat")
                        nc.scalar.activation(lat[:, :cn], psa[:, :cn],
                                             mybir.ActivationFunctionType.Lrelu,
                                             alpha=0.2)
                        pse = psE.tile([1, 512], F32, space="PSUM", tag="psE")
                        nc.tensor.matmul(pse[:, :cn], w2c_h, lat[:, :cn],
                                         start=True, stop=True)
                        ebuf = scp.tile([1, 512], F32, tag="ebuf")
                        if ch % 2 == 0:
                            nc.vector.tensor_copy(ebuf[:, :cn], pse[:, :cn])
                        else:
                            nc.scalar.copy(ebuf[:, :cn], pse[:, :cn])
                        nc.sync.dma_start(efd[0:1, c0:c0 + cn], ebuf[:, :cn])
                    # ---- softmax over groups of 12 ----
                    ng = n // SAMPLE
                    ef = efd[:]
                    if n == N2:
                        e1 = scp.tile([128, 72], F32, tag="e1")
                        nc.sync.dma_start(
                            e1[:], bass.AP(ef.tensor, ef.offset, [[72, 128], [1, 72]]))
                        nc.scalar.activation(e1[:], e1[:], mybir.ActivationFunctionType.Exp)
                        sg = scp.tile([128, 6], F32, tag="sg")
                        nc.vector.reduce_sum(
                            out=sg[:], in_=e1[:].rearrange("p (g s) -> p g s", s=SAMPLE),
                            axis=mybir.AxisListType.X)
                        nc.vector.reciprocal(sg[:], sg[:])
                        al = scp.tile([128, 72], F32, tag="al")
                        nc.vector.tensor_tensor(
                            out=al[:].rearrange("p (g s) -> p g s", s=SAMPLE),
                            in0=e1[:].rearrange("p (g s) -> p g s", s=SAMPLE),
                            in1=sg[:].to_broadcast([128, 6, SAMPLE]),
                            op=mybir.AluOpType.mult)
                        alf = drp.tile([1, N2], F32, tag="alf")
                        nc.sync.dma_start(
                            bass.AP(alf[:].tensor, alf[:].offset, [[72, 128], [1, 72]]),
                            al[:])
                    else:
                        e1 = scp.tile([64, 12], F32, tag="e1s")
                        nc.sync.dma_start(
                            e1[:], bass.AP(ef.tensor, ef.offset, [[12, 64], [1, 12]]))
                        nc.scalar.activation(e1[:], e1[:], mybir.ActivationFunctionType.Exp)
                        sg = scp.tile([64, 1], F32, tag="sgs")
                        nc.vector.reduce_sum(out=sg[:], in_=e1[:], axis=mybir.AxisListType.X)
                        nc.vector.reciprocal(sg[:], sg[:])
                        al = scp.tile([64, 12], F32, tag="als")
                        nc.vector.tensor_scalar(out=al[:], in0=e1[:], scalar1=sg[:],
                                                scalar2=None, op0=mybir.AluOpType.mult)
                        alf = drp.tile([1, N2], F32, tag="alf")
                        nc.sync.dma_start(
                            bass.AP(alf[:].tensor, alf[:].offset, [[12, 64], [1, 12]]),
                            al[:])
                    alt = scp.tile([72, 128], F32, tag="alt")
                    nc.sync.dma_start(
                        alt[:ncol, :],
                        bass.AP(alf[:].tensor, alf[:].offset, [[128, ncol], [1, 128]]))
                    psac = psA.tile([128, 512], F32, space="PSUM", tag="psA")
                    nc.tensor.transpose(psac[:, :ncol], alt[:ncol, :], identf[:ncol, :ncol])
                    alcol = scp.tile([128, 72], F32, tag="alcol")
                    nc.vector.tensor_copy(alcol[:, :ncol], psac[:, :ncol])
                    alg = scp.tile([128, 864], BF16, tag="alg")
                    nc.vector.tensor_tensor(
                        out=alg[:, :ncol * 12].rearrange("p (c s) -> p c s", s=12),
                        in0=pat_sb[:, :ncol * 12].rearrange("p (c s) -> p c s", s=12),
                        in1=alcol[:, :ncol].to_broadcast([128, ncol, 12]),
                        op=mybir.AluOpType.mult)
                    # ---- weighted sum: nvT[:, g] = sum rows*al ----
                    psnv = psNV.tile([128, 768], F32, space="PSUM", tag="psNV")
                    zb = scp.tile([1, 512], BF16, tag="zb")
                    nc.vector.memset(zb[:], 0.0)
                    nc.tensor.matmul(psnv[:, 0:min(512, ng)], zb[:, :128],
                                     zb[:, :min(512, ng)], start=True, stop=False)
                    if ng > 512:
                        nc.tensor.matmul(psnv[:, 512:ng], zb[:, :128], zb[:, :ng - 512],
                                         start=True, stop=False)
                    for c in range(ncol):
                        g0, gcnt = wr[c]
                        nc.tensor.matmul(
                            psnv[:, g0:g0 + gcnt],
                            rows[:].rearrange("p (c d) -> p c d", c=ncol)[:, c, :],
                            alg[:, c * 12:c * 12 + gcnt],
                            start=False, stop=False)
                    nc.tensor.matmul(psnv[:, 0:min(512, ng)], zb[:, :128],
                                     zb[:, :min(512, ng)], start=False, stop=True)
                    if ng > 512:
                        nc.tensor.matmul(psnv[:, 512:ng], zb[:, :128], zb[:, :ng - 512],
                                         start=False, stop=True)
                    nvt = scp.tile([128, 768], BF16, tag="nvt")
                    nc.vector.tensor_copy(nvt[:, :ng], psnv[:, :ng])
                    # ---- out = relu(w3a^T evT + w3b^T nvT) ----
                    psout = psNV.tile([128, 768], F32, space="PSUM", tag="psNV")
                    for ch in range((M + 511) // 512):
                        c0, cn = ch * 512, min(512, M - ch * 512)
                        nc.tensor.matmul(psout[:, c0:c0 + cn], w3a, evT[:, c0:c0 + cn],
                                         start=True, stop=False)
                        nc.tensor.matmul(psout[:, c0:c0 + cn], w3b, nvt[:, c0:c0 + cn],
                                         start=False, stop=True)
                    nc.scalar.activation(outt[:, :M], psout[:, :M],
                                         mybir.ActivationFunctionType.Relu)

                # ---------- combine h_local + h_global for this b ----------
                pb, cb = (b % 2) * 64, b // 2
                psF = psTb.tile([128, 128], BF16, space="PSUM", tag="psTb")
                nc.tensor.transpose(psF[:64, :], hgT[:], identb[:])
                ob = scp.tile([64, D], F32, tag="ob")
                nc.vector.tensor_tensor(
                    out=ob[:], in0=psF[:64, :],
                    in1=hl_sb[:].rearrange("p (c d) -> p c d", c=4)[pb:pb + 64, cb, :],
                    op=mybir.AluOpType.add)
                nc.sync.dma_start(t_out[b * 64:(b + 1) * 64, :], ob[:])
    nc.finalize()
    return nc


def _host_prep(inputs_np):
    inp = {k: np.asarray(v) for k, v in inputs_np.items()}
    emb = inp["embedding"].astype(np.float32)
    embb = emb.astype(ml_dtypes.bfloat16)
    adj_all = inp["adj_all"].astype(np.int64)
    num_w = inp["num_w"].astype(np.float32)
    ii = inp["inputs"].astype(np.int64)
    item = inp["item"].astype(np.int64)

    w1g = np.concatenate([inp["g0_w1"][:D], inp["g1_w1"][:D]], 1).astype(np.float32)
    w1d = np.concatenate([inp["g0_w1"][D], inp["g1_w1"][D]])[None, :].astype(ml_dtypes.bfloat16)
    w2c = np.stack([inp["g0_w2"], inp["g1_w2"]], 1).astype(ml_dtypes.bfloat16)
    w3 = np.concatenate([inp["g0_w3"][:D], inp["g0_w3"][D:], inp["g1_w3"][:D],
                         inp["g1_w3"][D:]], 1).astype(ml_dtypes.bfloat16)
    onesh = np.zeros((128, 2), np.float32)
    onesh[:64, 0] = 1.0
    onesh[64:, 1] = 1.0
    common = {
        "embf": emb, "embb": embb,
        "pat": _build_pattern(),
        "acol": np.stack([inp["a0"], inp["a1"], inp["a2"], inp["a3"]], 1).astype(np.float32),
        "onesh": onesh, "w1g": w1g, "w1d": w1d, "w2c": w2c, "w3": w3,
    }
    in_maps = []
    for c in range(NCORES):
        bsl = slice(c * BL, (c + 1) * BL)
        ii_c, item_c = ii[bsl], item[bsl]
        n1 = adj_all[ii_c].reshape(BL, N1)
        w1v = num_w[ii_c].reshape(BL, N1)
        n2 = adj_all[n1].reshape(BL, N2)
        w2v = num_w[n1].reshape(BL, N2)
        o1w = np.concatenate([_slotwrap(n1[b].astype(np.int32), 6) for b in range(BL)], 1)
        o2w = np.concatenate([_slotwrap(n2[b].astype(np.int32), 72) for b in range(BL)], 1)
        adj_c = inp["adj"][bsl].astype(np.int32)
        adjw = adj_c.reshape(4, 2, 64, 64).transpose(1, 2, 0, 3).reshape(128, 256)
        m = {
            "offh": _slotwrap(ii_c.reshape(-1).astype(np.int32), 4),
            "offi": _slotwrap(item_c.reshape(-1).astype(np.int32), 4),
            "maskf": _slotwrap(inp["mask_item"][bsl].reshape(-1).astype(np.float32), 4),
            "adjw": np.ascontiguousarray(adjw),
            "o1w": o1w, "o2w": o2w,
            "w1v": w1v.reshape(1, -1).astype(ml_dtypes.bfloat16),
            "w2v": w2v.reshape(1, -1).astype(ml_dtypes.bfloat16),
        }
        m.update(common)
        in_maps.append(m)
    return in_maps


def kernel(**inputs):
    if "nc" not in _compiled:
        _compiled["nc"] = _build_program()
    nc = _compiled["nc"]
    in_maps = _host_prep(inputs)
    res = run_bass_kernel_spmd(nc, in_maps, list(range(NCORES)))
    outs = [np.asarray(res.results[c]["out"]).reshape(BL, S, D) for c in range(NCORES)]
    return np.concatenate(outs, 0)


if __name__ == "__main__":
    pass


# revision 16
# speedup vs baseline: 2.9659x; 2.9659x over previous
"""Trainium2 Bass kernel for nn_CombineGraph (GNN message passing).

Strategy (8 NeuronCores, data-parallel over batch B=64 -> 8 per core):
  - Embedding table replicated per core. The dominant cost is gathering
    ~74K embedding rows per core; done with the Ant extended dma_gather
    (bf16, 256B rows). Its indices are int16 (max 32767) but V=50000, so
    the table is split into LO [0,32767) and HI [32767,50000) halves, each
    with a planted all-zero row; every slot is gathered from both halves
    (the "wrong" half yields the zero row) and the two results are added.
  - Row-major gathered tiles feed the per-group weighted-sum matmuls
    (attention application); a second SBUF-source transposing dma_gather
    (iota indices) produces the [D, n] layout for the d-contraction
    matmuls (the GlobalAggregator MLPs) on the PE in bf16.
  - Softmax over the 12 samples per group runs in a [groups, 12] layout
    produced by small AP-reshape DMAs; no max-subtraction is needed
    (logits are O(0.3) by construction).
  - The LocalAggregator (masked 4-relation GAT) runs in exact fp32.
  - Neighbor-sample index chains (adj_all[inputs] etc.) are pure integer
    lookups of host-visible inputs; they are precomputed host-side and
    shipped as gather offset tensors (input staging).
"""

import numpy as np
import ml_dtypes

import concourse.bass as bass
import concourse.bacc as bacc
import concourse.mybir as mybir
import concourse.tile as tile
from concourse.bass_utils import run_bass_kernel_spmd
from concourse.masks import make_identity

B, S, SAMPLE, D, V, HOP = 64, 64, 12, 128, 50000, 2
NCORES = 8
BL = B // NCORES          # 8 batches per core
N1 = S * SAMPLE           # 768
N2 = N1 * SAMPLE          # 9216
SPLIT = 32767             # LO covers [0, SPLIT); HI covers [SPLIT, V)
NHI = V - SPLIT + 1       # HI table rows (incl. zero row at 0)
F32 = mybir.dt.float32
BF16 = mybir.dt.bfloat16
I32 = mybir.dt.int32
I16 = mybir.dt.int16
BIG = 1.0e30

_compiled = {}


def _wrap16(v):
    """int16 idx buffer: [128, n/16]; buf[j, i] = v[16*i + j], replicated 8x."""
    n = v.shape[0]
    w = v.astype(np.int16).reshape(n // 16, 16).T
    return np.ascontiguousarray(np.tile(w, (8, 1)))


def _slotwrap(v, cols):
    """[cols*128] -> [128, cols] with slot r=(c*128+p) -> [p, c]."""
    return np.ascontiguousarray(v.reshape(cols, 128).T)


def _build_pattern():
    """PAT[128, 72*12] f32: PAT[p, c*12+k] = 1 iff (128c+p)//12 == (128c)//12 + k."""
    pat = np.zeros((128, 72 * 12), np.float32)
    for c in range(72):
        g0 = (128 * c) // 12
        for p in range(128):
            g = (128 * c + p) // 12
            pat[p, c * 12 + (g - g0)] = 1.0
    return pat


def _ws_ranges(ncol):
    """per tile-col c: (g0, gcnt) group range covered by rows 128c..128c+127."""
    out = []
    for c in range(ncol):
        g0 = (128 * c) // 12
        g1 = (128 * c + 127) // 12
        out.append((g0, g1 - g0 + 1))
    return out


def _build_program():
    nc = bacc.Bacc(None, target_bir_lowering=False)
    dt = nc.dram_tensor
    # ---- replicated tables / weights ----
    t_embf = dt("embf", [V, D], F32, kind="ExternalInput")
    t_embb = dt("embb", [V, D], BF16, kind="ExternalInput")
    t_pat = dt("pat", [128, 864], F32, kind="ExternalInput")
    t_acol = dt("acol", [128, 4], F32, kind="ExternalInput")
    t_onesh = dt("onesh", [128, 2], F32, kind="ExternalInput")
    t_w1g = dt("w1g", [128, 2 * D], F32, kind="ExternalInput")      # g0_w1[:D] | g1_w1[:D]
    t_w1d = dt("w1d", [1, 2 * D], BF16, kind="ExternalInput")       # g0_w1[D] | g1_w1[D]
    t_w2c = dt("w2c", [128, 2], BF16, kind="ExternalInput")         # g0_w2 | g1_w2
    t_w3 = dt("w3", [128, 4 * D], BF16, kind="ExternalInput")       # a0|b0|a1|b1 blocks
    # ---- per-core data ----
    t_offh = dt("offh", [128, 4], I32, kind="ExternalInput")
    t_offi = dt("offi", [128, 4], I32, kind="ExternalInput")
    t_maskf = dt("maskf", [128, 4], F32, kind="ExternalInput")
    t_adjw = dt("adjw", [128, 4 * 64], I32, kind="ExternalInput")
    t_o1 = dt("o1w", [128, 6 * BL], I32, kind="ExternalInput")
    t_o2 = dt("o2w", [128, 72 * BL], I32, kind="ExternalInput")
    t_w1v = dt("w1v", [1, BL * N1], BF16, kind="ExternalInput")
    t_w2v = dt("w2v", [1, BL * N2], BF16, kind="ExternalInput")
    t_out = dt("out", [BL * S, D], F32, kind="ExternalOutput")

    from contextlib import ExitStack
    with tile.TileContext(nc) as tc, ExitStack() as _es:
        cpool = _es.enter_context(tc.tile_pool(name="consts", bufs=1))
        pat_sb = cpool.tile([128, 864], F32)
        acol_sb = cpool.tile([128, 4], F32)
        onesh_sb = cpool.tile([128, 2], F32)
        w1g_sb = cpool.tile([128, 2 * D], F32)
        w1d_sb = cpool.tile([1, 2 * D], BF16)
        w2c_sb = cpool.tile([128, 2], BF16)
        w3_sb = cpool.tile([128, 4 * D], BF16)
        identf = cpool.tile([128, 128], F32)
        identb = cpool.tile([128, 128], BF16)
        offh_sb = cpool.tile([128, 4], I32)
        offi_sb = cpool.tile([128, 4], I32)
        maskf_sb = cpool.tile([128, 4], F32)
        adjw_sb = cpool.tile([128, 256], I32)
        o1_sb = cpool.tile([128, 6 * BL], I32)
        o2_sb = cpool.tile([128, 72 * BL], I32)
        for dst, src in [(pat_sb, t_pat), (acol_sb, t_acol),
                         (onesh_sb, t_onesh), (w1g_sb, t_w1g), (w1d_sb, t_w1d),
                         (w2c_sb, t_w2c), (w3_sb, t_w3), (offh_sb, t_offh),
                         (offi_sb, t_offi), (maskf_sb, t_maskf), (adjw_sb, t_adjw),
                         (o1_sb, t_o1), (o2_sb, t_o2)]:
            nc.sync.dma_start(dst[:], src[:])
        make_identity(nc, identf[:])
        make_identity(nc, identb[:])

        # persistent working tiles
        ppool = _es.enter_context(tc.tile_pool(name="persist", bufs=1))
        h_sb = ppool.tile([128, 4 * D], F32)          # h rows, slot layout
        item_sb = ppool.tile([128, 4 * D], F32)
        hT_sb = ppool.tile([128, BL * S], F32)        # hT per b at cols b*64..
        hTb_sb = ppool.tile([128, BL * S], BF16)
        hl_sb = ppool.tile([128, 4 * D], F32)         # h_local, slot layout
        w1s_sb = ppool.tile([128, 2 * BL * D], BF16)  # W1s per (hop, b)
        scol_sb = ppool.tile([128, BL], F32)          # s_b column vectors
        srow_sb = ppool.tile([2, 4 * D], F32)         # s_b row form (pack 2)
        recips_sb = ppool.tile([2, 4], F32)

        # ================= Phase 0: gathers + local aggregator =================
        with tc.tile_pool(name="p0psum1", bufs=1, space="PSUM") as ps0a, \
             tc.tile_pool(name="p0psum2", bufs=2, space="PSUM") as ps0, \
             tc.tile_pool(name="p0sb", bufs=2) as sb0:
            for c in range(4):
                nc.gpsimd.indirect_dma_start(
                    out=h_sb[:].rearrange("p (c d) -> p c d", c=4)[:, c, :],
                    out_offset=None, in_=t_embf[:],
                    in_offset=bass.IndirectOffsetOnAxis(ap=offh_sb[:, c:c + 1], axis=0))
                nc.gpsimd.indirect_dma_start(
                    out=item_sb[:].rearrange("p (c d) -> p c d", c=4)[:, c, :],
                    out_offset=None, in_=t_embf[:],
                    in_offset=bass.IndirectOffsetOnAxis(ap=offi_sb[:, c:c + 1], axis=0))

            # --- session summary s_b = masked mean of item rows ---
            mitem = sb0.tile([128, 4 * D], F32, tag="mitem")
            nc.vector.tensor_tensor(
                out=mitem[:].rearrange("p (c d) -> p c d", c=4),
                in0=item_sb[:].rearrange("p (c d) -> p c d", c=4),
                in1=maskf_sb[:].to_broadcast([128, 4, D]),
                op=mybir.AluOpType.mult)
            ps_s = ps0a.tile([2, 4 * D], F32, space="PSUM", tag="pss")
            ps_m = ps0a.tile([2, 4], F32, space="PSUM", tag="psm")
            nc.tensor.matmul(ps_m[:], onesh_sb[:], maskf_sb[:], start=True, stop=True)
            nc.tensor.matmul(ps_s[:], onesh_sb[:], mitem[:], start=True, stop=True)
            nc.vector.reciprocal(recips_sb[:], ps_m[:])
            nc.vector.tensor_tensor(
                out=srow_sb[:].rearrange("p (c d) -> p c d", c=4),
                in0=ps_s[:].rearrange("p (c d) -> p c d", c=4),
                in1=recips_sb[:].to_broadcast([2, 4, D]),
                op=mybir.AluOpType.mult)
            for b in range(BL):
                nc.sync.dma_start(
                    scol_sb[:, b:b + 1],
                    srow_sb[b % 2:b % 2 + 1, (b // 2) * D:(b // 2 + 1) * D])
            for hop in range(2):
                for b in range(BL):
                    nc.vector.tensor_scalar(
                        out=w1s_sb[:, (hop * BL + b) * D:(hop * BL + b + 1) * D],
                        in0=w1g_sb[:, hop * D:(hop + 1) * D],
                        scalar1=scol_sb[:, b:b + 1], scalar2=None,
                        op0=mybir.AluOpType.mult)

            # --- local aggregator, exact fp32, per b ---
            hb64s = []
            for b in range(BL):
                pb, cb = (b % 2) * 64, b // 2
                hrow = h_sb[:].rearrange("p (c d) -> p c d", c=4)[pb:pb + 64, cb, :]
                hb64 = sb0.tile([64, 128], F32, tag=f"hb64_{b}")
                nc.vector.tensor_copy(hb64[:], hrow)
                hb64s.append(hb64)
                psT = ps0.tile([128, 128], F32, space="PSUM", tag="psT")
                nc.tensor.transpose(psT[:, :64], hb64[:], identf[:64, :64])
                nc.vector.tensor_copy(hT_sb[:, b * 64:(b + 1) * 64], psT[:, :64])
                nc.scalar.copy(hTb_sb[:, b * 64:(b + 1) * 64], psT[:, :64])
            for b in range(BL):
                pb, cb = (b % 2) * 64, b // 2
                hT_b = hT_sb[:, b * 64:(b + 1) * 64]
                e_ps = ps0.tile([64, 256], F32, space="PSUM", tag="eps")
                hta = sb0.tile([128, 256], F32, tag="hta")
                for k in range(4):
                    nc.vector.tensor_scalar(
                        out=hta[:, k * 64:(k + 1) * 64], in0=hT_b,
                        scalar1=acol_sb[:, k:k + 1], scalar2=None,
                        op0=mybir.AluOpType.mult)
                for k in range(4):
                    nc.tensor.matmul(e_ps[:, k * 64:(k + 1) * 64],
                                     hta[:, k * 64:(k + 1) * 64], hT_b,
                                     start=True, stop=True)
                el = sb0.tile([64, 256], F32, tag="el")
                nc.scalar.activation(el[:], e_ps[:], mybir.ActivationFunctionType.Lrelu,
                                     alpha=0.2)
                adj_b = adjw_sb[pb:pb + 64, cb * 64:(cb + 1) * 64]
                mk = sb0.tile([64, 256], F32, tag="mk")
                for k in range(4):
                    nc.vector.tensor_scalar(out=mk[:, k * 64:(k + 1) * 64], in0=adj_b,
                                            scalar1=float(k + 1), scalar2=None,
                                            op0=mybir.AluOpType.is_equal)
                pr = sb0.tile([64, 256], F32, tag="pr")
                nc.vector.tensor_tensor(out=pr[:], in0=el[:], in1=mk[:],
                                        op=mybir.AluOpType.mult)
                alpha = sb0.tile([64, 64], F32, tag="alpha")
                nc.vector.tensor_tensor(out=alpha[:], in0=pr[:, 0:64], in1=pr[:, 64:128],
                                        op=mybir.AluOpType.add)
                nc.vector.tensor_tensor(out=alpha[:], in0=alpha[:], in1=pr[:, 128:192],
                                        op=mybir.AluOpType.add)
                nc.vector.tensor_tensor(out=alpha[:], in0=alpha[:], in1=pr[:, 192:256],
                                        op=mybir.AluOpType.add)
                pen = sb0.tile([64, 64], F32, tag="pen")
                nc.vector.tensor_scalar(out=pen[:], in0=adj_b, scalar1=0.0, scalar2=None,
                                        op0=mybir.AluOpType.is_gt)
                nc.vector.tensor_scalar(out=pen[:], in0=pen[:], scalar1=1.0, scalar2=BIG,
                                        op0=mybir.AluOpType.subtract,
                                        op1=mybir.AluOpType.mult)
                nc.vector.tensor_tensor(out=alpha[:], in0=alpha[:], in1=pen[:],
                                        op=mybir.AluOpType.add)
                ae = sb0.tile([64, 64], F32, tag="ae")
                nc.scalar.activation(ae[:], alpha[:], mybir.ActivationFunctionType.Exp)
                s1 = sb0.tile([64, 1], F32, tag="s1")
                nc.vector.reduce_sum(out=s1[:], in_=ae[:], axis=mybir.AxisListType.X)
                nc.vector.reciprocal(s1[:], s1[:])
                nc.vector.tensor_scalar(out=ae[:], in0=ae[:], scalar1=s1[:],
                                        scalar2=None, op0=mybir.AluOpType.mult)
                psT2 = ps0.tile([128, 128], F32, space="PSUM", tag="psT")
                nc.tensor.transpose(psT2[:64, :64], ae[:], identf[:64, :64])
                alphaT = sb0.tile([64, 64], F32, tag="alphaT")
                nc.vector.tensor_copy(alphaT[:], psT2[:64, :64])
                psHL = ps0.tile([64, 128], F32, space="PSUM", tag="psHL")
                nc.tensor.matmul(psHL[:], alphaT[:], hb64s[b][:],
                                 start=True, stop=True)
                nc.vector.tensor_copy(
                    hl_sb[:].rearrange("p (c d) -> p c d", c=4)[pb:pb + 64, cb, :], psHL[:])

        # ================= Phase 2: sampling gathers + global aggregator =======
        WR1 = _ws_ranges(6)
        WR2 = _ws_ranges(72)
        with tc.tile_pool(name="psA", bufs=2, space="PSUM") as psA, \
             tc.tile_pool(name="psE", bufs=2, space="PSUM") as psE, \
             tc.tile_pool(name="psNV", bufs=1, space="PSUM") as psNV, \
             tc.tile_pool(name="psTb", bufs=2, space="PSUM") as psTb, \
             tc.tile_pool(name="g1", bufs=2) as g1p, \
             tc.tile_pool(name="g2", bufs=2) as g2p, \
             tc.tile_pool(name="sc", bufs=3) as scp, \
             tc.tile_pool(name="drp", bufs=2, space="DRAM") as drp:
            for b in range(BL):
                # ---------- gathers for this b ----------
                r1lo = g1p.tile([128, 6 * D], BF16, tag="r1lo")
                r2lo = g1p.tile([128, 72 * D], BF16, tag="r2lo")
                rt1 = g2p.tile([128, N1], BF16, tag="rt1")
                rt2 = g2p.tile([128, N2], BF16, tag="rt2")
                for c in range(6):
                    nc.gpsimd.indirect_dma_start(
                        out=r1lo[:].rearrange("p (c d) -> p c d", c=6)[:, c, :],
                        out_offset=None, in_=t_embb[:],
                        in_offset=bass.IndirectOffsetOnAxis(
                            ap=o1_sb[:, b * 6 + c:b * 6 + c + 1], axis=0))
                for c in range(72):
                    nc.gpsimd.indirect_dma_start(
                        out=r2lo[:].rearrange("p (c d) -> p c d", c=72)[:, c, :],
                        out_offset=None, in_=t_embb[:],
                        in_offset=bass.IndirectOffsetOnAxis(
                            ap=o2_sb[:, b * 72 + c:b * 72 + c + 1], axis=0))
                nc.sync.dma_start(out=rt1[:].rearrange("p (c n) -> p c n", c=6),
                                  in_=r1lo[:].rearrange("p (c d) -> p c d", c=6),
                                  transpose=True)
                nc.sync.dma_start(out=rt2[:].rearrange("p (c n) -> p c n", c=72),
                                  in_=r2lo[:].rearrange("p (c d) -> p c d", c=72),
                                  transpose=True)

                # ---------- hop stages ----------
                ev1T = scp.tile([128, N1], BF16, tag="ev1T")      # relu out of h1
                ev0T = scp.tile([128, 64], BF16, tag="ev0T")      # relu out of h0
                r1g = scp.tile([128, 6 * D], BF16, tag="r1g")     # rows of ev1T
                hgT = scp.tile([128, 64], BF16, tag="hgT")

                stages = [
                    ("h0", 0, rt1, r1lo, t_w1v, b * N1, N1, 6, WR1, hTb_sb[:, b * 64:(b + 1) * 64], 64, ev0T),
                    ("h1", 0, rt2, r2lo, t_w2v, b * N2, N2, 72, WR2, rt1[:], N1, ev1T),
                    ("g1", 1, ev1T[:], r1g, t_w1v, b * N1, N1, 6, WR1, ev0T[:], 64, hgT),
                ]
                for (nm, hop, rtT, rows, wv, wv0, n, ncol, wr, evT, M, outt) in stages:
                    rtT_ap = rtT if isinstance(rtT, bass.AP) else rtT[:]
                    if nm == "g1":
                        # rows of ev1T via PE transposes
                        for tix in range(6):
                            psTT = psTb.tile([128, 128], BF16, space="PSUM", tag="psTb")
                            nc.tensor.transpose(psTT[:],
                                                ev1T[:, tix * 128:(tix + 1) * 128],
                                                identb[:])
                            nc.vector.tensor_copy(
                                r1g[:].rearrange("p (c d) -> p c d", c=6)[:, tix, :],
                                psTT[:])
                    w1s_b = w1s_sb[:, (hop * BL + b) * D:(hop * BL + b + 1) * D]
                    w1d_h = w1d_sb[:, hop * D:(hop + 1) * D]
                    w2c_h = w2c_sb[:, hop:hop + 1]
                    w3a = w3_sb[:, (2 * hop) * D:(2 * hop + 1) * D]
                    w3b = w3_sb[:, (2 * hop + 1) * D:(2 * hop + 2) * D]
                    # ---- A = W1s^T rtT + w1D (x) wv ; leaky ; e = w2^T LAT ----
                    efd = drp.tile([1, N2], F32, tag="efd")
                    nchunks = (n + 511) // 512
                    for ch in range(nchunks):
                        c0, cn = ch * 512, min(512, n - ch * 512)
                        wvc = scp.tile([1, 512], BF16, tag="wvc")
                        nc.sync.dma_start(wvc[:, :cn], wv[:, wv0 + c0:wv0 + c0 + cn])
                        psa = psA.tile([128, 512], F32, space="PSUM", tag="psA")
                        nc.tensor.matmul(psa[:, :cn], w1s_b, rtT_ap[:, c0:c0 + cn],
                                         start=True, stop=False)
                        nc.tensor.matmul(psa[:, :cn], w1d_h, wvc[:, :cn],
                                         start=False, stop=True)
                        lat = scp.tile([128, 512], BF16, tag="lat")
                        nc.scalar.activation(lat[:, :cn], psa[:, :cn],
                                             mybir.ActivationFunctionType.Lrelu,
                                             alpha=0.2)
                        pse = psE.tile([1, 512], F32, space="PSUM", tag="psE")
                        nc.tensor.matmul(pse[:, :cn], w2c_h, lat[:, :cn],
                                         start=True, stop=True)
                        ebuf = scp.tile([1, 512], F32, tag="ebuf")
                        if ch % 2 == 0:
                            nc.vector.tensor_copy(ebuf[:, :cn], pse[:, :cn])
                        else:
                            nc.scalar.copy(ebuf[:, :cn], pse[:, :cn])
                        nc.sync.dma_start(efd[0:1, c0:c0 + cn], ebuf[:, :cn])
                    # ---- softmax over groups of 12 ----
                    ng = n // SAMPLE
                    ef = efd[:]
                    if n == N2:
                        e1 = scp.tile([128, 72], F32, tag="e1")
                        nc.sync.dma_start(
                            e1[:], bass.AP(ef.tensor, ef.offset, [[72, 128], [1, 72]]))
                        nc.scalar.activation(e1[:], e1[:], mybir.ActivationFunctionType.Exp)
                        sg = scp.tile([128, 6], F32, tag="sg")
                        nc.vector.reduce_sum(
                            out=sg[:], in_=e1[:].rearrange("p (g s) -> p g s", s=SAMPLE),
                            axis=mybir.AxisListType.X)
                        nc.vector.reciprocal(sg[:], sg[:])
                        al = scp.tile([128, 72], F32, tag="al")
                        nc.vector.tensor_tensor(
                            out=al[:].rearrange("p (g s) -> p g s", s=SAMPLE),
                            in0=e1[:].rearrange("p (g s) -> p g s", s=SAMPLE),
                            in1=sg[:].to_broadcast([128, 6, SAMPLE]),
                            op=mybir.AluOpType.mult)
                        alf = drp.tile([1, N2], F32, tag="alf")
                        nc.sync.dma_start(
                            bass.AP(alf[:].tensor, alf[:].offset, [[72, 128], [1, 72]]),
                            al[:])
                    else:
                        e1 = scp.tile([64, 12], F32, tag="e1s")
                        nc.sync.dma_start(
                            e1[:], bass.AP(ef.tensor, ef.offset, [[12, 64], [1, 12]]))
                        nc.scalar.activation(e1[:], e1[:], mybir.ActivationFunctionType.Exp)
                        sg = scp.tile([64, 1], F32, tag="sgs")
                        nc.vector.reduce_sum(out=sg[:], in_=e1[:], axis=mybir.AxisListType.X)
                        nc.vector.reciprocal(sg[:], sg[:])
                        al = scp.tile([64, 12], F32, tag="als")
                        nc.vector.tensor_scalar(out=al[:], in0=e1[:], scalar1=sg[:],
                                                scalar2=None, op0=mybir.AluOpType.mult)
                        alf = drp.tile([1, N2], F32, tag="alf")
                        nc.sync.dma_start(
                            bass.AP(alf[:].tensor, alf[:].offset, [[12, 64], [1, 12]]),
                            al[:])
                    alt = scp.tile([72, 128], F32, tag="alt")
                    nc.sync.dma_start(
                        alt[:ncol, :],
                        bass.AP(alf[:].tensor, alf[:].offset, [[128, ncol], [1, 128]]))
                    psac = psA.tile([128, 512], F32, space="PSUM", tag="psA")
                    nc.tensor.transpose(psac[:, :ncol], alt[:ncol, :], identf[:ncol, :ncol])
                    alcol = scp.tile([128, 72], F32, tag="alcol")
                    nc.vector.tensor_copy(alcol[:, :ncol], psac[:, :ncol])
                    alg = scp.tile([128, 864], BF16, tag="alg")
                    nc.vector.tensor_tensor(
                        out=alg[:, :ncol * 12].rearrange("p (c s) -> p c s", s=12),
                        in0=pat_sb[:, :ncol * 12].rearrange("p (c s) -> p c s", s=12),
                        in1=alcol[:, :ncol].to_broadcast([128, ncol, 12]),
                        op=mybir.AluOpType.mult)
                    # ---- weighted sum: nvT[:, g] = sum rows*al ----
                    psnv = psNV.tile([128, 768], F32, space="PSUM", tag="psNV")
                    zb = scp.tile([1, 512], BF16, tag="zb")
                    nc.vector.memset(zb[:], 0.0)
                    nc.tensor.matmul(psnv[:, 0:min(512, ng)], zb[:, :128],
                                     zb[:, :min(512, ng)], start=True, stop=False)
                    if ng > 512:
                        nc.tensor.matmul(psnv[:, 512:ng], zb[:, :128], zb[:, :ng - 512],
                                         start=True, stop=False)
                    for c in range(ncol):
                        g0, gcnt = wr[c]
                        nc.tensor.matmul(
                            psnv[:, g0:g0 + gcnt],
                            rows[:].rearrange("p (c d) -> p c d", c=ncol)[:, c, :],
                            alg[:, c * 12:c * 12 + gcnt],
                            start=False, stop=False)
                    nc.tensor.matmul(psnv[:, 0:min(512, ng)], zb[:, :128],
                                     zb[:, :min(512, ng)], start=False, stop=True)
                    if ng > 512:
                        nc.tensor.matmul(psnv[:, 512:ng], zb[:, :128], zb[:, :ng - 512],
                                         start=False, stop=True)
                    nvt = scp.tile([128, 768], BF16, tag="nvt")
                    nc.vector.tensor_copy(nvt[:, :ng], psnv[:, :ng])
                    # ---- out = relu(w3a^T evT + w3b^T nvT) ----
                    psout = psNV.tile([128, 768], F32, space="PSUM", tag="psNV")
                    for ch in range((M + 511) // 512):
                        c0, cn = ch * 512, min(512, M - ch * 512)
                        nc.tensor.matmul(psout[:, c0:c0 + cn], w3a, evT[:, c0:c0 + cn],
                                         start=True, stop=False)
                        nc.tensor.matmul(psout[:, c0:c0 + cn], w3b, nvt[:, c0:c0 + cn],
                                         start=False, stop=True)
                    nc.scalar.activation(outt[:, :M], psout[:, :M],
                                         mybir.ActivationFunctionType.Relu)

                # ---------- combine h_local + h_global for this b ----------
                pb, cb = (b % 2) * 64, b // 2
                psF = psTb.tile([128, 128], BF16, space="PSUM", tag="psTb")
                nc.tensor.transpose(psF[:64, :], hgT[:], identb[:])
                ob = scp.tile([64, D], F32, tag="ob")
                nc.vector.tensor_tensor(
                    out=ob[:], in0=psF[:64, :],
                    in1=hl_sb[:].rearrange("p (c d) -> p c d", c=4)[pb:pb + 64, cb, :],
                    op=mybir.AluOpType.add)
                nc.sync.dma_start(t_out[b * 64:(b + 1) * 64, :], ob[:])
    nc.finalize()
    return nc


def _host_prep(inputs_np):
    inp = {k: np.asarray(v) for k, v in inputs_np.items()}
    emb = inp["embedding"].astype(np.float32)
    embb = emb.astype(ml_dtypes.bfloat16)
    adj_all = inp["adj_all"].astype(np.int64)
    num_w = inp["num_w"].astype(np.float32)
    ii = inp["inputs"].astype(np.int64)
    item = inp["item"].astype(np.int64)

    w1g = np.concatenate([inp["g0_w1"][:D], inp["g1_w1"][:D]], 1).astype(np.float32)
    w1d = np.concatenate([inp["g0_w1"][D], inp["g1_w1"][D]])[None, :].astype(ml_dtypes.bfloat16)
    w2c = np.stack([inp["g0_w2"], inp["g1_w2"]], 1).astype(ml_dtypes.bfloat16)
    w3 = np.concatenate([inp["g0_w3"][:D], inp["g0_w3"][D:], inp["g1_w3"][:D],
                         inp["g1_w3"][D:]], 1).astype(ml_dtypes.bfloat16)
    onesh = np.zeros((128, 2), np.float32)
    onesh[:64, 0] = 1.0
    onesh[64:, 1] = 1.0
    common = {
        "embf": emb, "embb": embb,
        "pat": _build_pattern(),
        "acol": np.stack([inp["a0"], inp["a1"], inp["a2"], inp["a3"]], 1).astype(np.float32),
        "onesh": onesh, "w1g": w1g, "w1d": w1d, "w2c": w2c, "w3": w3,
    }
    in_maps = []
    for c in range(NCORES):
        bsl = slice(c * BL, (c + 1) * BL)
        ii_c, item_c = ii[bsl], item[bsl]
        n1 = adj_all[ii_c].reshape(BL, N1)
        w1v = num_w[ii_c].reshape(BL, N1)
        n2 = adj_all[n1].reshape(BL, N2)
        w2v = num_w[n1].reshape(BL, N2)
        o1w = np.concatenate([_slotwrap(n1[b].astype(np.int32), 6) for b in range(BL)], 1)
        o2w = np.concatenate([_slotwrap(n2[b].astype(np.int32), 72) for b in range(BL)], 1)
        adj_c = inp["adj"][bsl].astype(np.int32)
        adjw = adj_c.reshape(4, 2, 64, 64).transpose(1, 2, 0, 3).reshape(128, 256)
        m = {
            "offh": _slotwrap(ii_c.reshape(-1).astype(np.int32), 4),
            "offi": _slotwrap(item_c.reshape(-1).astype(np.int32), 4),
            "maskf": _slotwrap(inp["mask_item"][bsl].reshape(-1).astype(np.float32), 4),
            "adjw": np.ascontiguousarray(adjw),
            "o1w": o1w, "o2w": o2w,
            "w1v": w1v.reshape(1, -1).astype(ml_dtypes.bfloat16),
            "w2v": w2v.reshape(1, -1).astype(ml_dtypes.bfloat16),
        }
        m.update(common)
        in_maps.append(m)
    return in_maps


def kernel(**inputs):
    if "nc" not in _compiled:
        _compiled["nc"] = _build_program()
    nc = _compiled["nc"]
    in_maps = _host_prep(inputs)
    res = run_bass_kernel_spmd(nc, in_maps, list(range(NCORES)))
    outs = [np.asarray(res.results[c]["out"]).reshape(BL, S, D) for c in range(NCORES)]
    return np.concatenate(outs, 0)


if __name__ == "__main__":
    pass
